# revision 2
# baseline (speedup 1.0000x reference)
"""EntAttentionLayer on 8 TRN2 NeuronCores — v2.

Sharding: pure sequence-parallel, no collectives. Core c handles batch
b = c//4 and query rows [qc*512, qc*512+512), qc = c%4. Each core
computes K/V for its batch's FULL sequence (redundant x4), its own 512
queries, and the whole per-row pipeline (SA -> CA -> FFN).

v2 changes vs baseline:
- bf16 operands for every matmul (same PE cost in cost model, half DMA).
- Projection/attention interleave: K/Q computed per head-pair right
  before that pair's scores, so Act-engine exp work starts early and
  overlaps the PE-dense projection phase.
- Scores for a head pair packed into one [128, 1024] PSUM tile -> one
  exp per (pair, kc) instead of two (halves Act instruction overhead).
- Softmax normalize inline per pair: reciprocal of the aug-V denominator
  row, DRAM-roundtrip partition-broadcast, fused normalize+copy into a
  pair-packed ctx layout [128, 6, SQ].
- CA softmax via the same aug-V denominator trick (no gpsimd
  partition_all_reduce, no pre-normalized probs).
- Out-projections contract 128 rows/chunk (head pairs) instead of 64.
- FF2 loops qt-outer with resident w2 so LN3 overlaps remaining matmuls.
- LN affine + cbo/b2 adds skipped when inputs are identity/zero.
"""
import sys
sys.path.insert(0, "/opt/trn_rl_repo")
import numpy as np
import ml_dtypes
import concourse.bass as bass
import concourse.mybir as mybir
import concourse.tile as tile
from concourse import bacc
from concourse import bass_utils

B, S, D, H, T, RAD = 2, 2048, 768, 12, 64, 50
DH = D // H          # 64
F = 4 * D            # 3072
SQ = S // 4          # 512 query rows per core
P = 128
NC = 8
HA = 65              # aug head width (64 ctx dims + 1 denom)
DA = H * HA          # 780
NP = H // 2          # 6 head pairs
BAND_COLS = [(0, 114), (14, 242), (142, 370), (270, 498), (398, 512)]
BAND_OFF = [0, 114, 342, 570, 798]
BAND_TOT = 912
F32 = mybir.dt.float32
BF16 = mybir.dt.bfloat16
I32 = mybir.dt.int32
AF = mybir.ActivationFunctionType
ALU = mybir.AluOpType
EPS = 1e-12

_CACHED = {}


def _rsqrt1(nc, pool, v1, magic):
    """DVE Newton rsqrt of v1 [P,1] (positive). Returns y [P,1]."""
    sh = pool.tile([P, 1], I32, name="rs_sh")
    nc.vector.tensor_scalar(out=sh[:], in0=v1.bitcast(I32), scalar1=1,
                            scalar2=None, op0=ALU.logical_shift_right)
    y = pool.tile([P, 1], F32, name="rs_y")
    nc.vector.tensor_tensor(y[:].bitcast(I32), magic[:], sh[:], ALU.subtract)
    t1 = pool.tile([P, 1], F32, name="rs_t1")
    for _ in range(2):
        nc.vector.tensor_mul(t1[:], v1, y[:])
        nc.vector.tensor_mul(t1[:], t1[:], y[:])
        nc.vector.tensor_scalar(out=t1[:], in0=t1[:], scalar1=-0.5,
                                scalar2=1.5, op0=ALU.mult, op1=ALU.add)
        nc.vector.tensor_mul(y[:], y[:], t1[:])
    return y


def _ln_mv(nc, pool, r_ap):
    """bn stats of r_ap [P, D] -> mv [P, 2] (mean, var)."""
    st = pool.tile([P, 3, 6], F32, name="ln_st")
    for g in range(3):
        nc.vector.bn_stats(st[:, g, :], r_ap[:, g * 256:(g + 1) * 256])
    mv = pool.tile([P, 2], F32, name="ln_mv")
    nc.vector.bn_aggr(mv[:], st[:])
    return mv


def build_kernel(aff=False, cbo_nz=False, b2_nz=False):
    nc = bacc.Bacc("TRN2", target_bir_lowering=False, debug=False,
                   num_devices=NC)

    def din(name, shape, dt=BF16):
        return nc.dram_tensor(name, shape, dt, kind="ExternalInput").ap()

    xT = din("xT", [D, S])                      # rotated hidden^T, bf16
    xres = din("xres", [SQ, D], F32)            # X rows + sa_bo
    m5 = din("mask5", [P, BAND_TOT], BF16)
    wq = din("wq", [D, D]);  bq = din("bq", [D], F32)     # pre-scaled 1/8
    wk = din("wk", [D, D]);  bk = din("bk", [D], F32)
    wv = din("wv", [D, DA]); bv_bc = din("bv_bc", [P, DA])
    wo = din("wo", [D, D])
    tagT = din("tagT", [D, T])
    cwq = din("cwq", [D, D]); cbq = din("cbq", [D], F32)  # pre-scaled 1/8
    cwk = din("cwk", [D, D]); cbk = din("cbk", [D], F32)
    cwv = din("cwv", [D, DA]); cbv_bc = din("cbv_bc", [T, DA])
    cwo = din("cwo", [D, D])
    w1 = din("w1", [D, F]); b1p = din("b1p", [P, F // P], F32)
    w2 = din("w2", [F, D])
    ident = din("ident", [P, P])
    ones_in = nc.dram_tensor("ones_in", [65, 64], BF16,
                             kind="ExternalInput").ap()
    if cbo_nz:
        cbo_bc = din("cbo_bc", [P, D], F32)
    if b2_nz:
        b2_bc = din("b2_bc", [P, D], F32)
    if aff:
        g1_bc = din("g1_bc", [P, D], F32); b1l_bc = din("b1l_bc", [P, D], F32)
        g2_bc = din("g2_bc", [P, D], F32); b2l_bc = din("b2l_bc", [P, D], F32)
        g3_bc = din("g3_bc", [P, D], F32); b3l_bc = din("b3l_bc", [P, D], F32)
    out = nc.dram_tensor("out", [SQ, D], F32, kind="ExternalOutput").ap()
    DBG = TRUNC = False
    if DBG:
        dbg_ctx2 = nc.dram_tensor("dbg_ctx2", [P, NP, SQ], BF16,
                                  kind="ExternalOutput").ap()
        dbg_a2 = nc.dram_tensor("dbg_a2", [P, 4, D], BF16,
                                kind="ExternalOutput").ap()
        dbg_kT2 = nc.dram_tensor("dbg_kT2", [P, 6, S], BF16,
                                 kind="ExternalOutput").ap()
        dbg_v2 = nc.dram_tensor("dbg_v2", [P, 16, DA], BF16,
                                kind="ExternalOutput").ap()
        dbg_q = nc.dram_tensor("dbg_q", [P, 6, SQ], F32,
                               kind="ExternalOutput").ap()
    F32R = mybir.dt.float32r

    with tile.TileContext(nc) as tc:
      with tc.tile_pool(name="consts", bufs=1) as consts:
        magic = consts.tile([P, 1], I32, name="magic")
        nc.vector.memset(magic[:], 0x5F3759DF)
        ones64 = consts.tile([65, 64], BF16, name="ones64")
        nc.sync.dma_start(ones64[:], ones_in)
        if aff:
            lng = []
            for nm, t in (("g1", g1_bc), ("b1l", b1l_bc), ("g2", g2_bc),
                          ("b2l", b2l_bc), ("g3", g3_bc), ("b3l", b3l_bc)):
                s = consts.tile([P, D], F32, name=nm)
                nc.sync.dma_start(s[:], t)
                lng.append(s)
            g1_sb, b1l_sb, g2_sb, b2l_sb, g3_sb, b3l_sb = lng

        def ln_apply(pool, r_ap, mv, rs, out_ap, gi):
            """out = (r - mean) * rstd [* g + b]."""
            if not aff:
                nc.vector.tensor_scalar(out=out_ap, in0=r_ap,
                                        scalar1=mv[:, 0:1], scalar2=rs[:],
                                        op0=ALU.subtract, op1=ALU.mult)
            else:
                g_sb = (g1_sb, g2_sb, g3_sb)[gi]
                b_sb = (b1l_sb, b2l_sb, b3l_sb)[gi]
                t = pool.tile([P, D], F32, name="ln_t")
                nc.vector.tensor_scalar(out=t[:], in0=r_ap,
                                        scalar1=mv[:, 0:1], scalar2=rs[:],
                                        op0=ALU.subtract, op1=ALU.mult)
                nc.vector.tensor_mul(t[:], t[:], g_sb[:])
                nc.vector.tensor_add(out_ap, t[:], b_sb[:])

        with tc.tile_pool(name="pers", bufs=1) as pers:
          ctxU2 = pers.tile([P, NP, SQ], BF16, name="ctxU2")
          aT_sb = pers.tile([P, 6, SQ], BF16, name="aT")
          a_sb = pers.tile([P, 4, D], BF16, name="a_sb")
          zT_sb = pers.tile([P, 6, SQ], BF16, name="zT")
          z_sb = pers.tile([P, 4, D], BF16, name="z_sb")

          # stage-3 inputs live in pers so their DMAs can run during SA
          # (not gated on the stage-2 pools' closure)
          wo_t = pers.tile([P, NP, D], BF16, name="wo_t")
          xres_sb = pers.tile([P, 4, D], F32, name="xres")
          tagT_sb = pers.tile([P, 6, T], BF16, name="tagT")
          cwk_t = pers.tile([P, 6, D], BF16, name="cwk_t")
          cwq_t = pers.tile([P, 6, D], BF16, name="cwq_t")
          cbv_sb = pers.tile([T, DA], BF16, name="cbv")

          # ---------------- Stage 2: projections + self-attention ------
          with tc.tile_pool(name="xtp", bufs=1) as xtp, \
               tc.tile_pool(name="wst", bufs=1) as wst:
            wv_t = wst.tile([P, 6, DA], BF16, name="wv_t")
            nc.sync.dma_start(wv_t[:], wv.rearrange("(c p) e -> p c e", p=P))
            xT_sb = xtp.tile([P, 6, S], BF16, name="xT")
            for cc in range(6):
                nc.sync.dma_start(
                    xT_sb[:, cc, :],
                    xT.rearrange("(c p) s -> p c s", p=P)[:, cc, :])
            bv_sb = xtp.tile([P, DA], BF16, name="bv")
            nc.sync.dma_start(bv_sb[:], bv_bc)
            wk_t = wst.tile([P, 6, D], BF16, name="wk_t")
            nc.sync.dma_start(wk_t[:], wk.rearrange("(c p) e -> p c e", p=P))
            wq_t = wst.tile([P, 6, D], BF16, name="wq_t")
            nc.sync.dma_start(wq_t[:], wq.rearrange("(c p) e -> p c e", p=P))
            bq_sb = consts.tile([P, 6], F32, name="bq")
            nc.sync.dma_start(bq_sb[:], bq.rearrange("(c p) -> p c", p=P))
            bk_sb = consts.tile([P, 6], F32, name="bk")
            nc.sync.dma_start(bk_sb[:], bk.rearrange("(c p) -> p c", p=P))
            cbq_sb = consts.tile([P, 6], F32, name="cbq")
            nc.sync.dma_start(cbq_sb[:], cbq.rearrange("(c p) -> p c", p=P))
            cbk_sb = consts.tile([P, 6], F32, name="cbk")
            nc.sync.dma_start(cbk_sb[:], cbk.rearrange("(c p) -> p c", p=P))
            ident_sb = consts.tile([P, P], BF16, name="ident")
            nc.sync.dma_start(ident_sb[:], ident)
            nc.sync.dma_start(wo_t[:], wo.rearrange("(c p) e -> p c e", p=P))
            nc.sync.dma_start(xres_sb[:],
                              xres.rearrange("(q p) e -> p q e", p=P))
            nc.sync.dma_start(tagT_sb[:],
                              tagT.rearrange("(c p) t -> p c t", p=P))
            nc.sync.dma_start(cwk_t[:],
                              cwk.rearrange("(c p) e -> p c e", p=P))
            nc.sync.dma_start(cwq_t[:],
                              cwq.rearrange("(c p) e -> p c e", p=P))
            nc.sync.dma_start(cbv_sb[:], cbv_bc)

            with tc.tile_pool(name="kvp", bufs=1) as kvp, \
                 tc.tile_pool(name="m5p", bufs=1) as m5p, \
                 tc.tile_pool(name="ep", bufs=3) as epool, \
                 tc.tile_pool(name="dnp", bufs=3) as dnp, \
                 tc.tile_pool(name="rbp", bufs=3) as rbp, \
                 tc.tile_pool(name="pj", bufs=2, space="PSUM") as pj, \
                 tc.tile_pool(name="scs", bufs=2, space="PSUM") as scs, \
                 tc.tile_pool(name="cxp", bufs=2, space="PSUM") as cxp:
              kT_sb = kvp.tile([P, 6, S], BF16, name="kT")
              qT_sb = kvp.tile([P, 6, SQ], BF16, name="qT")
              v_sb = kvp.tile([P, 16, DA], BF16, name="v")
              m5_sb = m5p.tile([P, BAND_TOT], BF16, name="m5")
              nc.sync.dma_start(m5_sb[:], m5)

              # V projection, full sequence (two column groups per sc so
              # the shared 1-bank PSUM tag double-buffers cleanly)
              for sc in range(16):
                  for lo, hi in ((0, SQ), (SQ, DA)):
                      psv = pj.tile([P, SQ], F32, name="ps_pj")
                      for cc in range(6):
                          nc.tensor.matmul(
                              psv[:, 0:hi - lo],
                              xT_sb[:, cc, sc * P:(sc + 1) * P],
                              wv_t[:, cc, lo:hi],
                              start=(cc == 0), stop=(cc == 5))
                      nc.vector.tensor_add(v_sb[:, sc, lo:hi],
                                           psv[:, 0:hi - lo],
                                           bv_sb[:, lo:hi])

              # head-pair loop: K/Q chunk then attention for that pair
              for pg in range(NP):
                  ha, hb = 2 * pg, 2 * pg + 1
                  for scc in range(4):
                      psk = pj.tile([P, SQ], F32, name="ps_pj")
                      for cc in range(6):
                          nc.tensor.matmul(
                              psk[:], wk_t[:, cc, pg * P:(pg + 1) * P],
                              xT_sb[:, cc, scc * SQ:(scc + 1) * SQ],
                              start=(cc == 0), stop=(cc == 5))
                      nc.vector.tensor_scalar(
                          out=kT_sb[:, pg, scc * SQ:(scc + 1) * SQ],
                          in0=psk[:], scalar1=bk_sb[:, pg:pg + 1],
                          scalar2=None, op0=ALU.add)
                  psq = pj.tile([P, SQ], F32, name="ps_pj")
                  for cc in range(6):
                      nc.tensor.matmul(psq[:],
                                       wq_t[:, cc, pg * P:(pg + 1) * P],
                                       xT_sb[:, cc, 64:64 + SQ],
                                       start=(cc == 0), stop=(cc == 5))
                  nc.vector.tensor_scalar(out=qT_sb[:, pg, :], in0=psq[:],
                                          scalar1=bq_sb[:, pg:pg + 1],
                                          scalar2=None, op0=ALU.add)

                  ctxA = cxp.tile([HA, SQ], F32, name="ctx")
                  ctxB = cxp.tile([HA, SQ], F32, name="ctx")
                  for kc in range(16):
                      s = scs.tile([P, 2 * SQ], F32, name="s")
                      nc.tensor.matmul(
                          s[:, 0:SQ], kT_sb[0:64, pg, kc * P:(kc + 1) * P],
                          qT_sb[0:64, pg, :], start=True, stop=True)
                      nc.tensor.matmul(
                          s[:, SQ:2 * SQ],
                          kT_sb[64:P, pg, kc * P:(kc + 1) * P],
                          qT_sb[64:P, pg, :], start=True, stop=True)
                      e = epool.tile([P, 2 * SQ], BF16, name="e")
                      nc.scalar.activation(e[:], s[:], AF.Exp)
                      if kc < 5:
                          lo, hi = BAND_COLS[kc]
                          mo = BAND_OFF[kc]
                          for off in (0, SQ):
                              nc.vector.tensor_tensor(
                                  e[:, off + lo:off + hi],
                                  e[:, off + lo:off + hi],
                                  m5_sb[:, mo:mo + hi - lo], ALU.mult)
                      nc.tensor.matmul(
                          ctxA[:], v_sb[:, kc, ha * HA:(ha + 1) * HA],
                          e[:, 0:SQ], start=(kc == 0), stop=(kc == 15))
                      nc.tensor.matmul(
                          ctxB[:], v_sb[:, kc, hb * HA:(hb + 1) * HA],
                          e[:, SQ:2 * SQ], start=(kc == 0), stop=(kc == 15))
                  for hh, cx, prow in ((ha, ctxA, 0), (hb, ctxB, 64)):
                      ds = dnp.tile([65, SQ], BF16, name="ds")
                      nc.vector.tensor_copy(ds[64:65, :], cx[64:65, :])
                      rbps = scs.tile([64, SQ], F32, name="s")
                      nc.tensor.matmul(rbps[:], ones64[64:65, :],
                                       ds[64:65, :], start=True, stop=True)
                      rb = rbp.tile([64, SQ], F32, name="rb")
                      nc.vector.reciprocal_approx_fast(rb[:], rbps[:])
                      nc.vector.tensor_tensor(
                          ctxU2[prow:prow + 64, pg, :], cx[0:64, :],
                          rb[:], ALU.mult)

          if DBG:
              nc.sync.dma_start(dbg_ctx2, ctxU2[:])

          # ---------------- Stages 1, 3, 4, 5 ----------------
          with tc.tile_pool(name="w1p", bufs=1) as w1p, \
               tc.tile_pool(name="w2p", bufs=1) as w2p:
            with tc.tile_pool(name="st3w", bufs=1) as st3w, \
                 tc.tile_pool(name="caw", bufs=1) as caw:
              # ---- Stage 1: tag-table K/V ----
              cwv_t = caw.tile([P, 6, DA], BF16, name="cwv_t")
              nc.sync.dma_start(cwv_t[:],
                                cwv.rearrange("(c p) e -> p c e", p=P))
              kca_sb = caw.tile([P, 6, T], BF16, name="kca")
              vca_sb = caw.tile([T, DA], BF16, name="vca")
              with tc.tile_pool(name="pjc", bufs=2, space="PSUM") as pjc:
                  for dc in range(6):
                      psc = pjc.tile([P, T], F32, name="ps_c")
                      for cc in range(6):
                          nc.tensor.matmul(psc[:],
                                           cwk_t[:, cc, dc * P:(dc + 1) * P],
                                           tagT_sb[:, cc, :],
                                           start=(cc == 0), stop=(cc == 5))
                      nc.vector.tensor_scalar(out=kca_sb[:, dc, :],
                                              in0=psc[:],
                                              scalar1=cbk_sb[:, dc:dc + 1],
                                              scalar2=None, op0=ALU.add)
                  psa = pjc.tile([T, SQ], F32, name="ps_ca")
                  psb = pjc.tile([T, DA - SQ], F32, name="ps_cb")
                  for cc in range(6):
                      nc.tensor.matmul(psa[:], tagT_sb[:, cc, :],
                                       cwv_t[:, cc, 0:SQ],
                                       start=(cc == 0), stop=(cc == 5))
                      nc.tensor.matmul(psb[:], tagT_sb[:, cc, :],
                                       cwv_t[:, cc, SQ:DA],
                                       start=(cc == 0), stop=(cc == 5))
                  nc.vector.tensor_add(vca_sb[:, 0:SQ], psa[:],
                                       cbv_sb[:, 0:SQ])
                  nc.vector.tensor_add(vca_sb[:, SQ:DA], psb[:],
                                       cbv_sb[:, SQ:DA])

              # ---------- Stage 3: SA out-proj, LN1, A^T ----------
              with tc.tile_pool(name="st3", bufs=2) as st3, \
                   tc.tile_pool(name="lnp", bufs=2) as lnp, \
                   tc.tile_pool(name="pso", bufs=2, space="PSUM") as pso, \
                   tc.tile_pool(name="pst", bufs=2, space="PSUM") as pst:
                  for qt in range(4):
                      po = pso.tile([P, D], F32, name="po")
                      for pgg in range(NP):
                          nc.tensor.matmul(
                              po[:, 0:512],
                              ctxU2[:, pgg, qt * P:(qt + 1) * P],
                              wo_t[:, pgg, 0:512],
                              start=(pgg == 0), stop=(pgg == NP - 1))
                          nc.tensor.matmul(
                              po[:, 512:D],
                              ctxU2[:, pgg, qt * P:(qt + 1) * P],
                              wo_t[:, pgg, 512:D],
                              start=(pgg == 0), stop=(pgg == NP - 1))
                      r = st3.tile([P, D], F32, name="r3")
                      nc.vector.tensor_add(r[:], xres_sb[:, qt, :], po[:])
                      mv = _ln_mv(nc, lnp, r[:])
                      var = lnp.tile([P, 1], F32, name="var")
                      nc.vector.tensor_scalar(out=var[:], in0=mv[:, 1:2],
                                              scalar1=EPS, scalar2=None,
                                              op0=ALU.add)
                      rs = _rsqrt1(nc, lnp, var[:], magic)
                      ln_apply(lnp, r[:], mv, rs, a_sb[:, qt, :], 0)
                      for ec in range(6):
                          pt = pst.tile([P, P], BF16, name="pt")
                          nc.tensor.transpose(
                              pt[:], a_sb[:, qt, ec * P:(ec + 1) * P],
                              ident_sb)
                          nc.scalar.copy(aT_sb[:, ec, qt * P:(qt + 1) * P],
                                         pt[:])

              if DBG:
                  nc.sync.dma_start(dbg_a2, a_sb[:])

              # ---------- Stage 4: cross-attention, LN2, Z^T ----------
              cwo_t = st3w.tile([P, NP, D], BF16, name="cwo_t")
              nc.sync.dma_start(cwo_t[:],
                                cwo.rearrange("(c p) e -> p c e", p=P))
              if cbo_nz:
                  cbo_sb = st3w.tile([P, D], F32, name="cbo")
                  nc.sync.dma_start(cbo_sb[:], cbo_bc)
              with tc.tile_pool(name="st4", bufs=2) as st4, \
                   tc.tile_pool(name="lnp4", bufs=2) as lnp4:
                qcaT_sb = st4.tile([P, 6, SQ], BF16, name="qcaT")
                with tc.tile_pool(name="ps4", bufs=2, space="PSUM") as ps4:
                    for dc in range(6):
                        ps = ps4.tile([P, SQ], F32, name="ps4t")
                        for cc in range(6):
                            nc.tensor.matmul(
                                ps[:], cwq_t[:, cc, dc * P:(dc + 1) * P],
                                aT_sb[:, cc, :],
                                start=(cc == 0), stop=(cc == 5))
                        nc.vector.tensor_scalar(out=qcaT_sb[:, dc, :],
                                                in0=ps[:],
                                                scalar1=cbq_sb[:, dc:dc + 1],
                                                scalar2=None, op0=ALU.add)
                with tc.tile_pool(name="s4p", bufs=2, space="PSUM") as s4p, \
                     tc.tile_pool(name="cx4", bufs=3, space="PSUM") as cx4, \
                     tc.tile_pool(name="ep4", bufs=3) as ep4, \
                     tc.tile_pool(name="dnp4", bufs=3) as dnp4, \
                     tc.tile_pool(name="rbp4", bufs=3) as rbp4:
                    for pg in range(NP):
                        ha, hb = 2 * pg, 2 * pg + 1
                        s4 = s4p.tile([T, 2 * SQ], F32, name="s4t")
                        nc.tensor.matmul(s4[:, 0:SQ], kca_sb[0:64, pg, :],
                                         qcaT_sb[0:64, pg, :],
                                         start=True, stop=True)
                        nc.tensor.matmul(s4[:, SQ:2 * SQ],
                                         kca_sb[64:P, pg, :],
                                         qcaT_sb[64:P, pg, :],
                                         start=True, stop=True)
                        e4 = ep4.tile([T, 2 * SQ], BF16, name="e4")
                        nc.scalar.activation(e4[:], s4[:], AF.Exp)
                        for hh, off, prow in ((ha, 0, 0), (hb, SQ, 64)):
                            cx = cx4.tile([HA, SQ], F32, name="cx4t")
                            nc.tensor.matmul(
                                cx[:], vca_sb[:, hh * HA:(hh + 1) * HA],
                                e4[:, off:off + SQ], start=True, stop=True)
                            rden = dnp4.tile([1, SQ], F32, name="rdn4")
                            nc.vector.reciprocal_approx_fast(rden[:],
                                                             cx[64:65, :])
                            rdenr = dnp4.tile([1, SQ], F32R, name="rdnr4")
                            nc.vector.tensor_copy(rdenr[:], rden[:])
                            rbps = s4p.tile([64, SQ], F32, name="s4t")
                            nc.tensor.matmul(rbps[:], ones1[:], rdenr[:],
                                             start=True, stop=True)
                            rb = rbp4.tile([64, SQ], BF16, name="rb4")
                            nc.scalar.copy(rb[:], rbps[:])
                            nc.vector.tensor_tensor(
                                ctxU2[prow:prow + 64, pg, :], cx[0:64, :],
                                rb[:], ALU.mult)
                with tc.tile_pool(name="pso4", bufs=2, space="PSUM") as pso4, \
                     tc.tile_pool(name="pst4", bufs=2, space="PSUM") as pst4:
                    for qt in range(4):
                        po = pso4.tile([P, D], F32, name="po4")
                        for pgg in range(NP):
                            nc.tensor.matmul(
                                po[:, 0:512],
                                ctxU2[:, pgg, qt * P:(qt + 1) * P],
                                cwo_t[:, pgg, 0:512],
                                start=(pgg == 0), stop=(pgg == NP - 1))
                            nc.tensor.matmul(
                                po[:, 512:D],
                                ctxU2[:, pgg, qt * P:(qt + 1) * P],
                                cwo_t[:, pgg, 512:D],
                                start=(pgg == 0), stop=(pgg == NP - 1))
                        r = st4.tile([P, D], F32, name="r4")
                        nc.vector.tensor_add(r[:], a_sb[:, qt, :], po[:])
                        if cbo_nz:
                            nc.vector.tensor_add(r[:], r[:], cbo_sb[:])
                        mv = _ln_mv(nc, lnp4, r[:])
                        var = lnp4.tile([P, 1], F32, name="var4")
                        nc.vector.tensor_scalar(out=var[:], in0=mv[:, 1:2],
                                                scalar1=EPS, scalar2=None,
                                                op0=ALU.add)
                        rs = _rsqrt1(nc, lnp4, var[:], magic)
                        ln_apply(lnp4, r[:], mv, rs, z_sb[:, qt, :], 1)
                        for ec in range(6):
                            pt = pst4.tile([P, P], BF16, name="pt4")
                            nc.tensor.transpose(
                                pt[:], z_sb[:, qt, ec * P:(ec + 1) * P],
                                ident_sb)
                            nc.scalar.copy(
                                zT_sb[:, ec, qt * P:(qt + 1) * P], pt[:])

            # ---------- Stage 5: FFN + LN3 + output ----------
            with tc.tile_pool(name="st5", bufs=2) as st5, \
                 tc.tile_pool(name="lnp5", bufs=2) as lnp5, \
                 tc.tile_pool(name="igp", bufs=1) as igp:
                b1p_sb = st5.tile([P, F // P, 1], F32, name="b1p")
                nc.sync.dma_start(b1p_sb[:], b1p[:, :, None])
                if b2_nz:
                    b2r_sb = st5.tile([P, D], F32, name="b2r")
                    nc.sync.dma_start(b2r_sb[:], b2_bc)
                ig_sb = igp.tile([P, F // P, SQ], BF16, name="ig")
                w1_t = w1p.tile([P, 6, F], BF16, name="w1_t")
                for cc in range(6):
                    for sh_ in range(2):
                        nc.sync.dma_start(
                            w1_t[:, cc, sh_ * 1536:(sh_ + 1) * 1536],
                            w1.rearrange("(c p) e -> p c e", p=P)[
                                :, cc, sh_ * 1536:(sh_ + 1) * 1536])
                w2_t = w2p.tile([P, F // P, D], BF16, name="w2_t")
                for fq in range(12):
                    nc.sync.dma_start(
                        w2_t[:, fq * 2:(fq + 1) * 2, :],
                        w2.rearrange("(c p) e -> p c e", p=P)[
                            :, fq * 2:(fq + 1) * 2, :])
                with tc.tile_pool(name="ps5", bufs=3, space="PSUM") as ps5, \
                     tc.tile_pool(name="pso5", bufs=2, space="PSUM") as pso5:
                    for fc in range(F // P):
                        ps = ps5.tile([P, SQ], F32, name="ps5t")
                        for cc in range(6):
                            nc.tensor.matmul(
                                ps[:], w1_t[:, cc, fc * P:(fc + 1) * P],
                                zT_sb[:, cc, :],
                                start=(cc == 0), stop=(cc == 5))
                        nc.scalar.activation(ig_sb[:, fc, :], ps[:], AF.Gelu,
                                             bias=b1p_sb[:, fc, 0:1])
                    for qt in range(4):
                        po5 = pso5.tile([P, D], F32, name="po5")
                        for fc in range(F // P):
                            nc.tensor.matmul(
                                po5[:, 0:512],
                                ig_sb[:, fc, qt * P:(qt + 1) * P],
                                w2_t[:, fc, 0:512],
                                start=(fc == 0), stop=(fc == F // P - 1))
                            nc.tensor.matmul(
                                po5[:, 512:D],
                                ig_sb[:, fc, qt * P:(qt + 1) * P],
                                w2_t[:, fc, 512:D],
                                start=(fc == 0), stop=(fc == F // P - 1))
                        r5 = st5.tile([P, D], F32, name="r5")
                        nc.vector.tensor_add(r5[:], z_sb[:, qt, :], po5[:])
                        if b2_nz:
                            nc.vector.tensor_add(r5[:], r5[:], b2r_sb[:])
                        mv = _ln_mv(nc, lnp5, r5[:])
                        var = lnp5.tile([P, 1], F32, name="var5")
                        nc.vector.tensor_scalar(out=var[:], in0=mv[:, 1:2],
                                                scalar1=EPS, scalar2=None,
                                                op0=ALU.add)
                        rs = _rsqrt1(nc, lnp5, var[:], magic)
                        o_sb = lnp5.tile([P, D], F32, name="o5")
                        ln_apply(lnp5, r5[:], mv, rs, o_sb[:], 2)
                        nc.sync.dma_start(out[qt * P:(qt + 1) * P, :],
                                          o_sb[:])

    nc.compile()
    return nc


def _prep_shared(inp):
    """Host-side shared (core-independent) arrays."""
    f32, bf = np.float32, ml_dtypes.bfloat16
    sh = {}
    sh["wq"] = np.ascontiguousarray((inp["sa_wq"] * 0.125).astype(bf))
    sh["bq"] = np.ascontiguousarray(inp["sa_bq"] * 0.125)
    sh["wk"] = np.ascontiguousarray(inp["sa_wk"].astype(bf))
    sh["bk"] = np.ascontiguousarray(inp["sa_bk"])

    def aug(wvm, bvm):
        wva = np.zeros((D, DA), f32)
        bva = np.zeros((DA,), f32)
        for h in range(H):
            wva[:, h * HA:h * HA + DH] = wvm[:, h * DH:(h + 1) * DH]
            bva[h * HA:h * HA + DH] = bvm[h * DH:(h + 1) * DH]
            bva[h * HA + DH] = 1.0
        return wva, bva

    wva, bva = aug(inp["sa_wv"], inp["sa_bv"])
    sh["wv"] = np.ascontiguousarray(wva.astype(bf))
    sh["bv_bc"] = np.ascontiguousarray(
        np.broadcast_to(bva, (P, DA)).astype(bf))
    sh["wo"] = np.ascontiguousarray(inp["sa_wo"].astype(bf))
    sh["tagT"] = np.ascontiguousarray(inp["tag_emb"].T.astype(bf))
    sh["cwq"] = np.ascontiguousarray((inp["ca_wq"] * 0.125).astype(bf))
    sh["cbq"] = np.ascontiguousarray(inp["ca_bq"] * 0.125)
    sh["cwk"] = np.ascontiguousarray(inp["ca_wk"].astype(bf))
    sh["cbk"] = np.ascontiguousarray(inp["ca_bk"])
    cwva, cbva = aug(inp["ca_wv"], inp["ca_bv"])
    sh["cwv"] = np.ascontiguousarray(cwva.astype(bf))
    sh["cbv_bc"] = np.ascontiguousarray(
        np.broadcast_to(cbva, (T, DA)).astype(bf))
    sh["cwo"] = np.ascontiguousarray(inp["ca_wo"].astype(bf))
    sh["w1"] = np.ascontiguousarray(inp["ff_w1"].astype(bf))
    sh["b1p"] = np.ascontiguousarray(inp["ff_b1"].reshape(F // P, P).T)
    sh["w2"] = np.ascontiguousarray(inp["ff_w2"].astype(bf))
    sh["ident"] = np.eye(P, dtype=f32).astype(bf)
    o64 = np.zeros((65, 64), f32)
    o64[64, :] = 1.0
    sh["ones_in"] = np.ascontiguousarray(o64.astype(bf))
    return sh


def _mask5_for(qc):
    q0 = qc * SQ
    pos = np.arange(5 * P)
    s_true = (pos - 64 + q0) % S
    u = np.arange(SQ)
    band = (np.abs((q0 + u)[None, :] - s_true[:, None]) <= RAD)
    bexp = np.where(band, np.float32(np.e), np.float32(1.0)).astype(np.float32)
    bexp = bexp.reshape(5, P, SQ).transpose(1, 0, 2)  # [P, 5, SQ]
    packed = np.empty((P, BAND_TOT), ml_dtypes.bfloat16)
    for j, (lo, hi) in enumerate(BAND_COLS):
        packed[:, BAND_OFF[j]:BAND_OFF[j] + hi - lo] = bexp[:, j, lo:hi]
    return np.ascontiguousarray(packed)


def _make_in_maps(inp, aff, cbo_nz, b2_nz):
    sh = _prep_shared(inp)
    bf = ml_dtypes.bfloat16
    if cbo_nz:
        sh["cbo_bc"] = np.ascontiguousarray(
            np.broadcast_to(inp["ca_bo"], (P, D)).astype(np.float32))
    if b2_nz:
        sh["b2_bc"] = np.ascontiguousarray(
            np.broadcast_to(inp["ff_b2"], (P, D)).astype(np.float32))
    if aff:
        for k, src in (("g1_bc", "sa_ln_g"), ("b1l_bc", "sa_ln_b"),
                       ("g2_bc", "ca_ln_g"), ("b2l_bc", "ca_ln_b"),
                       ("g3_bc", "ff_ln_g"), ("b3l_bc", "ff_ln_b")):
            sh[k] = np.ascontiguousarray(
                np.broadcast_to(inp[src], (P, D)).astype(np.float32))
    masks = [_mask5_for(qc) for qc in range(4)]
    hs = inp["hidden_states"]
    in_maps = []
    for c in range(NC):
        b, qc = c // 4, c % 4
        q0 = qc * SQ
        xTb = hs[b].T
        m = dict(sh)
        m["xT"] = np.ascontiguousarray(
            np.roll(xTb, 64 - q0, axis=1).astype(bf))
        m["xres"] = np.ascontiguousarray(hs[b, q0:q0 + SQ] + inp["sa_bo"])
        m["mask5"] = masks[qc]
        in_maps.append(m)
    return in_maps


def kernel(**inputs):
    inp = {k: np.asarray(v, dtype=np.float32) for k, v in inputs.items()}
    aff = not (np.all(inp["sa_ln_g"] == 1) and np.all(inp["sa_ln_b"] == 0)
               and np.all(inp["ca_ln_g"] == 1) and np.all(inp["ca_ln_b"] == 0)
               and np.all(inp["ff_ln_g"] == 1) and np.all(inp["ff_ln_b"] == 0))
    cbo_nz = bool(np.any(inp["ca_bo"] != 0))
    b2_nz = bool(np.any(inp["ff_b2"] != 0))
    key = (aff, cbo_nz, b2_nz)
    if key not in _CACHED:
        _CACHED[key] = build_kernel(aff, cbo_nz, b2_nz)
    nc = _CACHED[key]

    in_maps = _make_in_maps(inp, aff, cbo_nz, b2_nz)
    res = bass_utils.run_bass_kernel_spmd(nc, in_maps, core_ids=list(range(NC)))
    out = np.empty((B, S, D), np.float32)
    for c in range(NC):
        b, qc = c // 4, c % 4
        out[b, qc * SQ:(qc + 1) * SQ] = res.results[c]["out"]
    return out


# revision 3
# speedup vs baseline: 1.0040x; 1.0040x over previous
"""EntAttentionLayer on 8 TRN2 NeuronCores — v2.

Sharding: pure sequence-parallel, no collectives. Core c handles batch
b = c//4 and query rows [qc*512, qc*512+512), qc = c%4. Each core
computes K/V for its batch's FULL sequence (redundant x4), its own 512
queries, and the whole per-row pipeline (SA -> CA -> FFN).

v2 changes vs baseline:
- bf16 operands for every matmul (same PE cost in cost model, half DMA).
- Projection/attention interleave: K/Q computed per head-pair right
  before that pair's scores, so Act-engine exp work starts early and
  overlaps the PE-dense projection phase.
- Scores for a head pair packed into one [128, 1024] PSUM tile -> one
  exp per (pair, kc) instead of two (halves Act instruction overhead).
- Softmax normalize inline per pair: reciprocal of the aug-V denominator
  row, DRAM-roundtrip partition-broadcast, fused normalize+copy into a
  pair-packed ctx layout [128, 6, SQ].
- CA softmax via the same aug-V denominator trick (no gpsimd
  partition_all_reduce, no pre-normalized probs).
- Out-projections contract 128 rows/chunk (head pairs) instead of 64.
- FF2 loops qt-outer with resident w2 so LN3 overlaps remaining matmuls.
- LN affine + cbo/b2 adds skipped when inputs are identity/zero.
"""
import sys
sys.path.insert(0, "/opt/trn_rl_repo")
import numpy as np
import ml_dtypes
import concourse.bass as bass
import concourse.mybir as mybir
import concourse.tile as tile
from concourse import bacc
from concourse import bass_utils

B, S, D, H, T, RAD = 2, 2048, 768, 12, 64, 50
DH = D // H          # 64
F = 4 * D            # 3072
SQ = S // 4          # 512 query rows per core
P = 128
NC = 8
HA = 65              # aug head width (64 ctx dims + 1 denom)
DA = H * HA          # 780
NP = H // 2          # 6 head pairs
BAND_COLS = [(0, 114), (14, 242), (142, 370), (270, 498), (398, 512)]
BAND_OFF = [0, 114, 342, 570, 798]
BAND_TOT = 912
F32 = mybir.dt.float32
BF16 = mybir.dt.bfloat16
I32 = mybir.dt.int32
AF = mybir.ActivationFunctionType
ALU = mybir.AluOpType
EPS = 1e-12

_CACHED = {}


def _rsqrt1(nc, pool, v1, magic):
    """DVE Newton rsqrt of v1 [P,1] (positive). Returns y [P,1]."""
    sh = pool.tile([P, 1], I32, name="rs_sh")
    nc.vector.tensor_scalar(out=sh[:], in0=v1.bitcast(I32), scalar1=1,
                            scalar2=None, op0=ALU.logical_shift_right)
    y = pool.tile([P, 1], F32, name="rs_y")
    nc.vector.tensor_tensor(y[:].bitcast(I32), magic[:], sh[:], ALU.subtract)
    t1 = pool.tile([P, 1], F32, name="rs_t1")
    for _ in range(2):
        nc.vector.tensor_mul(t1[:], v1, y[:])
        nc.vector.tensor_mul(t1[:], t1[:], y[:])
        nc.vector.tensor_scalar(out=t1[:], in0=t1[:], scalar1=-0.5,
                                scalar2=1.5, op0=ALU.mult, op1=ALU.add)
        nc.vector.tensor_mul(y[:], y[:], t1[:])
    return y


def _ln_mv(nc, pool, r_ap):
    """bn stats of r_ap [P, D] -> mv [P, 2] (mean, var)."""
    st = pool.tile([P, 3, 6], F32, name="ln_st")
    for g in range(3):
        nc.vector.bn_stats(st[:, g, :], r_ap[:, g * 256:(g + 1) * 256])
    mv = pool.tile([P, 2], F32, name="ln_mv")
    nc.vector.bn_aggr(mv[:], st[:])
    return mv


def build_kernel(aff=False, cbo_nz=False, b2_nz=False):
    nc = bacc.Bacc("TRN2", target_bir_lowering=False, debug=False,
                   num_devices=NC)

    def din(name, shape, dt=BF16):
        return nc.dram_tensor(name, shape, dt, kind="ExternalInput").ap()

    xT = din("xT", [D, S])                      # rotated hidden^T, bf16
    xres = din("xres", [SQ, D], F32)            # X rows + sa_bo
    m5 = din("mask5", [P, BAND_TOT], BF16)
    wq = din("wq", [D, D]);  bq = din("bq", [D], F32)     # pre-scaled 1/8
    wk = din("wk", [D, D]);  bk = din("bk", [D], F32)
    wv = din("wv", [D, DA]); bv_bc = din("bv_bc", [P, DA])
    wo = din("wo", [D, D])
    tagT = din("tagT", [D, T])
    cwq = din("cwq", [D, D]); cbq = din("cbq", [D], F32)  # pre-scaled 1/8
    cwk = din("cwk", [D, D]); cbk = din("cbk", [D], F32)
    cwv = din("cwv", [D, DA]); cbv_bc = din("cbv_bc", [T, DA])
    cwo = din("cwo", [D, D])
    w1 = din("w1", [D, F]); b1p = din("b1p", [P, F // P], F32)
    w2 = din("w2", [F, D])
    ident = din("ident", [P, P])
    ones_in = nc.dram_tensor("ones_in", [65, 64], BF16,
                             kind="ExternalInput").ap()
    if cbo_nz:
        cbo_bc = din("cbo_bc", [P, D], F32)
    if b2_nz:
        b2_bc = din("b2_bc", [P, D], F32)
    if aff:
        g1_bc = din("g1_bc", [P, D], F32); b1l_bc = din("b1l_bc", [P, D], F32)
        g2_bc = din("g2_bc", [P, D], F32); b2l_bc = din("b2l_bc", [P, D], F32)
        g3_bc = din("g3_bc", [P, D], F32); b3l_bc = din("b3l_bc", [P, D], F32)
    out = nc.dram_tensor("out", [SQ, D], F32, kind="ExternalOutput").ap()
    DBG = TRUNC = False
    if DBG:
        dbg_ctx2 = nc.dram_tensor("dbg_ctx2", [P, NP, SQ], BF16,
                                  kind="ExternalOutput").ap()
        dbg_a2 = nc.dram_tensor("dbg_a2", [P, 4, D], BF16,
                                kind="ExternalOutput").ap()
        dbg_kT2 = nc.dram_tensor("dbg_kT2", [P, 6, S], BF16,
                                 kind="ExternalOutput").ap()
        dbg_v2 = nc.dram_tensor("dbg_v2", [P, 16, DA], BF16,
                                kind="ExternalOutput").ap()
        dbg_q = nc.dram_tensor("dbg_q", [P, 6, SQ], F32,
                               kind="ExternalOutput").ap()
    F32R = mybir.dt.float32r

    with tile.TileContext(nc) as tc:
      with tc.tile_pool(name="consts", bufs=1) as consts:
        magic = consts.tile([P, 1], I32, name="magic")
        nc.vector.memset(magic[:], 0x5F3759DF)
        ones64 = consts.tile([65, 64], BF16, name="ones64")
        nc.sync.dma_start(ones64[:], ones_in)
        if aff:
            lng = []
            for nm, t in (("g1", g1_bc), ("b1l", b1l_bc), ("g2", g2_bc),
                          ("b2l", b2l_bc), ("g3", g3_bc), ("b3l", b3l_bc)):
                s = consts.tile([P, D], F32, name=nm)
                nc.sync.dma_start(s[:], t)
                lng.append(s)
            g1_sb, b1l_sb, g2_sb, b2l_sb, g3_sb, b3l_sb = lng

        def ln_apply(pool, r_ap, mv, rs, out_ap, gi):
            """out = (r - mean) * rstd [* g + b]."""
            if not aff:
                nc.vector.tensor_scalar(out=out_ap, in0=r_ap,
                                        scalar1=mv[:, 0:1], scalar2=rs[:],
                                        op0=ALU.subtract, op1=ALU.mult)
            else:
                g_sb = (g1_sb, g2_sb, g3_sb)[gi]
                b_sb = (b1l_sb, b2l_sb, b3l_sb)[gi]
                t = pool.tile([P, D], F32, name="ln_t")
                nc.vector.tensor_scalar(out=t[:], in0=r_ap,
                                        scalar1=mv[:, 0:1], scalar2=rs[:],
                                        op0=ALU.subtract, op1=ALU.mult)
                nc.vector.tensor_mul(t[:], t[:], g_sb[:])
                nc.vector.tensor_add(out_ap, t[:], b_sb[:])

        with tc.tile_pool(name="pers", bufs=1) as pers:
          ctxU2 = pers.tile([P, NP, SQ], BF16, name="ctxU2")
          aT_sb = pers.tile([P, 6, SQ], BF16, name="aT")
          a_sb = pers.tile([P, 4, D], BF16, name="a_sb")
          zT_sb = pers.tile([P, 6, SQ], BF16, name="zT")
          z_sb = pers.tile([P, 4, D], BF16, name="z_sb")

          # stage-3 inputs live in pers so their DMAs can run during SA
          # (not gated on the stage-2 pools' closure)
          wo_t = pers.tile([P, NP, D], BF16, name="wo_t")
          xres_sb = pers.tile([P, 4, D], F32, name="xres")
          tagT_sb = pers.tile([P, 6, T], BF16, name="tagT")
          cwk_t = pers.tile([P, 6, D], BF16, name="cwk_t")
          cwq_t = pers.tile([P, 6, D], BF16, name="cwq_t")
          cbv_sb = pers.tile([T, DA], BF16, name="cbv")

          # ---------------- Stage 2: projections + self-attention ------
          with tc.tile_pool(name="xtp", bufs=1) as xtp, \
               tc.tile_pool(name="wst", bufs=1) as wst:
            wv_t = wst.tile([P, 6, DA], BF16, name="wv_t")
            nc.sync.dma_start(wv_t[:], wv.rearrange("(c p) e -> p c e", p=P))
            xT_sb = xtp.tile([P, 6, S], BF16, name="xT")
            for cc in range(6):
                nc.sync.dma_start(
                    xT_sb[:, cc, :],
                    xT.rearrange("(c p) s -> p c s", p=P)[:, cc, :])
            bv_sb = xtp.tile([P, DA], BF16, name="bv")
            nc.sync.dma_start(bv_sb[:], bv_bc)
            wk_t = wst.tile([P, 6, D], BF16, name="wk_t")
            nc.sync.dma_start(wk_t[:], wk.rearrange("(c p) e -> p c e", p=P))
            wq_t = wst.tile([P, 6, D], BF16, name="wq_t")
            nc.sync.dma_start(wq_t[:], wq.rearrange("(c p) e -> p c e", p=P))
            bq_sb = consts.tile([P, 6], F32, name="bq")
            nc.sync.dma_start(bq_sb[:], bq.rearrange("(c p) -> p c", p=P))
            bk_sb = consts.tile([P, 6], F32, name="bk")
            nc.sync.dma_start(bk_sb[:], bk.rearrange("(c p) -> p c", p=P))
            cbq_sb = consts.tile([P, 6], F32, name="cbq")
            nc.sync.dma_start(cbq_sb[:], cbq.rearrange("(c p) -> p c", p=P))
            cbk_sb = consts.tile([P, 6], F32, name="cbk")
            nc.sync.dma_start(cbk_sb[:], cbk.rearrange("(c p) -> p c", p=P))
            ident_sb = consts.tile([P, P], BF16, name="ident")
            nc.sync.dma_start(ident_sb[:], ident)
            nc.sync.dma_start(wo_t[:], wo.rearrange("(c p) e -> p c e", p=P))
            nc.sync.dma_start(xres_sb[:],
                              xres.rearrange("(q p) e -> p q e", p=P))
            nc.sync.dma_start(tagT_sb[:],
                              tagT.rearrange("(c p) t -> p c t", p=P))
            nc.sync.dma_start(cwk_t[:],
                              cwk.rearrange("(c p) e -> p c e", p=P))
            nc.sync.dma_start(cwq_t[:],
                              cwq.rearrange("(c p) e -> p c e", p=P))
            nc.sync.dma_start(cbv_sb[:], cbv_bc)

            with tc.tile_pool(name="kvp", bufs=1) as kvp, \
                 tc.tile_pool(name="m5p", bufs=1) as m5p, \
                 tc.tile_pool(name="ep", bufs=3) as epool, \
                 tc.tile_pool(name="dnp", bufs=3) as dnp, \
                 tc.tile_pool(name="rbp", bufs=3) as rbp, \
                 tc.tile_pool(name="pj", bufs=2, space="PSUM") as pj, \
                 tc.tile_pool(name="scs", bufs=2, space="PSUM") as scs, \
                 tc.tile_pool(name="cxp", bufs=2, space="PSUM") as cxp:
              kT_sb = kvp.tile([P, 6, S], BF16, name="kT")
              qT_sb = kvp.tile([P, 6, SQ], BF16, name="qT")
              v_sb = kvp.tile([P, 16, DA], BF16, name="v")
              m5_sb = m5p.tile([P, BAND_TOT], BF16, name="m5")
              nc.sync.dma_start(m5_sb[:], m5)

              # V projection, full sequence (two column groups per sc so
              # the shared 1-bank PSUM tag double-buffers cleanly)
              for sc in range(16):
                  for lo, hi in ((0, SQ), (SQ, DA)):
                      psv = pj.tile([P, SQ], F32, name="ps_pj")
                      for cc in range(6):
                          nc.tensor.matmul(
                              psv[:, 0:hi - lo],
                              xT_sb[:, cc, sc * P:(sc + 1) * P],
                              wv_t[:, cc, lo:hi],
                              start=(cc == 0), stop=(cc == 5))
                      nc.vector.tensor_add(v_sb[:, sc, lo:hi],
                                           psv[:, 0:hi - lo],
                                           bv_sb[:, lo:hi])

              # head-pair loop: K/Q chunk then attention for that pair
              for pg in range(NP):
                  ha, hb = 2 * pg, 2 * pg + 1
                  for scc in range(4):
                      psk = pj.tile([P, SQ], F32, name="ps_pj")
                      for cc in range(6):
                          nc.tensor.matmul(
                              psk[:], wk_t[:, cc, pg * P:(pg + 1) * P],
                              xT_sb[:, cc, scc * SQ:(scc + 1) * SQ],
                              start=(cc == 0), stop=(cc == 5))
                      nc.vector.tensor_scalar(
                          out=kT_sb[:, pg, scc * SQ:(scc + 1) * SQ],
                          in0=psk[:], scalar1=bk_sb[:, pg:pg + 1],
                          scalar2=None, op0=ALU.add)
                  psq = pj.tile([P, SQ], F32, name="ps_pj")
                  for cc in range(6):
                      nc.tensor.matmul(psq[:],
                                       wq_t[:, cc, pg * P:(pg + 1) * P],
                                       xT_sb[:, cc, 64:64 + SQ],
                                       start=(cc == 0), stop=(cc == 5))
                  nc.vector.tensor_scalar(out=qT_sb[:, pg, :], in0=psq[:],
                                          scalar1=bq_sb[:, pg:pg + 1],
                                          scalar2=None, op0=ALU.add)

                  ctxA = cxp.tile([HA, SQ], F32, name="ctx")
                  ctxB = cxp.tile([HA, SQ], F32, name="ctx")
                  for kc in range(16):
                      s = scs.tile([P, 2 * SQ], F32, name="s")
                      nc.tensor.matmul(
                          s[:, 0:SQ], kT_sb[0:64, pg, kc * P:(kc + 1) * P],
                          qT_sb[0:64, pg, :], start=True, stop=True)
                      nc.tensor.matmul(
                          s[:, SQ:2 * SQ],
                          kT_sb[64:P, pg, kc * P:(kc + 1) * P],
                          qT_sb[64:P, pg, :], start=True, stop=True)
                      e = epool.tile([P, 2 * SQ], BF16, name="e")
                      nc.scalar.activation(e[:], s[:], AF.Exp)
                      if kc < 5:
                          lo, hi = BAND_COLS[kc]
                          mo = BAND_OFF[kc]
                          for off in (0, SQ):
                              nc.vector.tensor_tensor(
                                  e[:, off + lo:off + hi],
                                  e[:, off + lo:off + hi],
                                  m5_sb[:, mo:mo + hi - lo], ALU.mult)
                      nc.tensor.matmul(
                          ctxA[:], v_sb[:, kc, ha * HA:(ha + 1) * HA],
                          e[:, 0:SQ], start=(kc == 0), stop=(kc == 15))
                      nc.tensor.matmul(
                          ctxB[:], v_sb[:, kc, hb * HA:(hb + 1) * HA],
                          e[:, SQ:2 * SQ], start=(kc == 0), stop=(kc == 15))
                  for hh, cx, prow in ((ha, ctxA, 0), (hb, ctxB, 64)):
                      ds = dnp.tile([65, SQ], BF16, name="ds")
                      nc.vector.tensor_copy(ds[64:65, :], cx[64:65, :])
                      rbps = scs.tile([64, SQ], F32, name="s")
                      nc.tensor.matmul(rbps[:], ones64[64:65, :],
                                       ds[64:65, :], start=True, stop=True)
                      rb = rbp.tile([64, SQ], F32, name="rb")
                      nc.vector.reciprocal_approx_fast(rb[:], rbps[:])
                      nc.vector.tensor_tensor(
                          ctxU2[prow:prow + 64, pg, :], cx[0:64, :],
                          rb[:], ALU.mult)

          if DBG:
              nc.sync.dma_start(dbg_ctx2, ctxU2[:])

          # ---------------- Stages 1, 3, 4, 5 ----------------
          with tc.tile_pool(name="w1p", bufs=1) as w1p, \
               tc.tile_pool(name="w2p", bufs=1) as w2p:
            with tc.tile_pool(name="st3w", bufs=1) as st3w, \
                 tc.tile_pool(name="caw", bufs=1) as caw:
              # ---- Stage 1: tag-table K/V ----
              cwv_t = caw.tile([P, 6, DA], BF16, name="cwv_t")
              nc.sync.dma_start(cwv_t[:],
                                cwv.rearrange("(c p) e -> p c e", p=P))
              kca_sb = caw.tile([P, 6, T], BF16, name="kca")
              vca_sb = caw.tile([T, DA], BF16, name="vca")
              with tc.tile_pool(name="pjc", bufs=2, space="PSUM") as pjc:
                  for dc in range(6):
                      psc = pjc.tile([P, T], F32, name="ps_c")
                      for cc in range(6):
                          nc.tensor.matmul(psc[:],
                                           cwk_t[:, cc, dc * P:(dc + 1) * P],
                                           tagT_sb[:, cc, :],
                                           start=(cc == 0), stop=(cc == 5))
                      nc.vector.tensor_scalar(out=kca_sb[:, dc, :],
                                              in0=psc[:],
                                              scalar1=cbk_sb[:, dc:dc + 1],
                                              scalar2=None, op0=ALU.add)
                  psa = pjc.tile([T, SQ], F32, name="ps_ca")
                  psb = pjc.tile([T, DA - SQ], F32, name="ps_cb")
                  for cc in range(6):
                      nc.tensor.matmul(psa[:], tagT_sb[:, cc, :],
                                       cwv_t[:, cc, 0:SQ],
                                       start=(cc == 0), stop=(cc == 5))
                      nc.tensor.matmul(psb[:], tagT_sb[:, cc, :],
                                       cwv_t[:, cc, SQ:DA],
                                       start=(cc == 0), stop=(cc == 5))
                  nc.vector.tensor_add(vca_sb[:, 0:SQ], psa[:],
                                       cbv_sb[:, 0:SQ])
                  nc.vector.tensor_add(vca_sb[:, SQ:DA], psb[:],
                                       cbv_sb[:, SQ:DA])

              # ---------- Stage 3: SA out-proj, LN1, A^T ----------
              with tc.tile_pool(name="st3", bufs=2) as st3, \
                   tc.tile_pool(name="lnp", bufs=2) as lnp, \
                   tc.tile_pool(name="pso", bufs=2, space="PSUM") as pso, \
                   tc.tile_pool(name="pst", bufs=2, space="PSUM") as pst:
                  for qt in range(4):
                      po = pso.tile([P, D], F32, name="po")
                      for pgg in range(NP):
                          nc.tensor.matmul(
                              po[:, 0:512],
                              ctxU2[:, pgg, qt * P:(qt + 1) * P],
                              wo_t[:, pgg, 0:512],
                              start=(pgg == 0), stop=(pgg == NP - 1))
                          nc.tensor.matmul(
                              po[:, 512:D],
                              ctxU2[:, pgg, qt * P:(qt + 1) * P],
                              wo_t[:, pgg, 512:D],
                              start=(pgg == 0), stop=(pgg == NP - 1))
                      r = st3.tile([P, D], F32, name="r3")
                      nc.vector.tensor_add(r[:], xres_sb[:, qt, :], po[:])
                      mv = _ln_mv(nc, lnp, r[:])
                      var = lnp.tile([P, 1], F32, name="var")
                      nc.vector.tensor_scalar(out=var[:], in0=mv[:, 1:2],
                                              scalar1=EPS, scalar2=None,
                                              op0=ALU.add)
                      rs = _rsqrt1(nc, lnp, var[:], magic)
                      ln_apply(lnp, r[:], mv, rs, a_sb[:, qt, :], 0)
                      for ec in range(6):
                          pt = pst.tile([P, P], BF16, name="pt")
                          nc.tensor.transpose(
                              pt[:], a_sb[:, qt, ec * P:(ec + 1) * P],
                              ident_sb)
                          nc.scalar.copy(aT_sb[:, ec, qt * P:(qt + 1) * P],
                                         pt[:])

              if DBG:
                  nc.sync.dma_start(dbg_a2, a_sb[:])

              # ---------- Stage 4: cross-attention, LN2, Z^T ----------
              cwo_t = st3w.tile([P, NP, D], BF16, name="cwo_t")
              nc.sync.dma_start(cwo_t[:],
                                cwo.rearrange("(c p) e -> p c e", p=P))
              if cbo_nz:
                  cbo_sb = st3w.tile([P, D], F32, name="cbo")
                  nc.sync.dma_start(cbo_sb[:], cbo_bc)
              with tc.tile_pool(name="st4", bufs=2) as st4, \
                   tc.tile_pool(name="lnp4", bufs=2) as lnp4:
                qcaT_sb = st4.tile([P, 6, SQ], BF16, name="qcaT")
                with tc.tile_pool(name="ps4", bufs=2, space="PSUM") as ps4:
                    for dc in range(6):
                        ps = ps4.tile([P, SQ], F32, name="ps4t")
                        for cc in range(6):
                            nc.tensor.matmul(
                                ps[:], cwq_t[:, cc, dc * P:(dc + 1) * P],
                                aT_sb[:, cc, :],
                                start=(cc == 0), stop=(cc == 5))
                        nc.scalar.activation(qcaT_sb[:, dc, :], ps[:],
                                             AF.Copy,
                                             bias=cbq_sb[:, dc:dc + 1])
                with tc.tile_pool(name="s4p", bufs=2, space="PSUM") as s4p, \
                     tc.tile_pool(name="cx4", bufs=3, space="PSUM") as cx4, \
                     tc.tile_pool(name="ep4", bufs=3) as ep4, \
                     tc.tile_pool(name="dnp4", bufs=3) as dnp4, \
                     tc.tile_pool(name="rbp4", bufs=3) as rbp4:
                    for pg in range(NP):
                        ha, hb = 2 * pg, 2 * pg + 1
                        s4 = s4p.tile([T, 2 * SQ], F32, name="s4t")
                        nc.tensor.matmul(s4[:, 0:SQ], kca_sb[0:64, pg, :],
                                         qcaT_sb[0:64, pg, :],
                                         start=True, stop=True)
                        nc.tensor.matmul(s4[:, SQ:2 * SQ],
                                         kca_sb[64:P, pg, :],
                                         qcaT_sb[64:P, pg, :],
                                         start=True, stop=True)
                        e4 = ep4.tile([T, 2 * SQ], BF16, name="e4")
                        nc.scalar.activation(e4[:], s4[:], AF.Exp)
                        for hh, off, prow in ((ha, 0, 0), (hb, SQ, 64)):
                            cx = cx4.tile([HA, SQ], F32, name="cx4t")
                            nc.tensor.matmul(
                                cx[:], vca_sb[:, hh * HA:(hh + 1) * HA],
                                e4[:, off:off + SQ], start=True, stop=True)
                            rden = dnp4.tile([1, SQ], F32, name="rdn4")
                            nc.vector.reciprocal_approx_fast(rden[:],
                                                             cx[64:65, :])
                            rdenr = dnp4.tile([1, SQ], F32R, name="rdnr4")
                            nc.vector.tensor_copy(rdenr[:], rden[:])
                            rbps = s4p.tile([64, SQ], F32, name="s4t")
                            nc.tensor.matmul(rbps[:], ones1[:], rdenr[:],
                                             start=True, stop=True)
                            rb = rbp4.tile([64, SQ], BF16, name="rb4")
                            nc.scalar.copy(rb[:], rbps[:])
                            nc.vector.tensor_tensor(
                                ctxU2[prow:prow + 64, pg, :], cx[0:64, :],
                                rb[:], ALU.mult)
                with tc.tile_pool(name="pso4", bufs=2, space="PSUM") as pso4, \
                     tc.tile_pool(name="pst4", bufs=2, space="PSUM") as pst4:
                    for qt in range(4):
                        po = pso4.tile([P, D], F32, name="po4")
                        for pgg in range(NP):
                            nc.tensor.matmul(
                                po[:, 0:512],
                                ctxU2[:, pgg, qt * P:(qt + 1) * P],
                                cwo_t[:, pgg, 0:512],
                                start=(pgg == 0), stop=(pgg == NP - 1))
                            nc.tensor.matmul(
                                po[:, 512:D],
                                ctxU2[:, pgg, qt * P:(qt + 1) * P],
                                cwo_t[:, pgg, 512:D],
                                start=(pgg == 0), stop=(pgg == NP - 1))
                        r = st4.tile([P, D], F32, name="r4")
                        nc.vector.tensor_add(r[:], a_sb[:, qt, :], po[:])
                        if cbo_nz:
                            nc.vector.tensor_add(r[:], r[:], cbo_sb[:])
                        mv = _ln_mv(nc, lnp4, r[:])
                        var = lnp4.tile([P, 1], F32, name="var4")
                        nc.vector.tensor_scalar(out=var[:], in0=mv[:, 1:2],
                                                scalar1=EPS, scalar2=None,
                                                op0=ALU.add)
                        rs = _rsqrt1(nc, lnp4, var[:], magic)
                        ln_apply(lnp4, r[:], mv, rs, z_sb[:, qt, :], 1)
                        for ec in range(6):
                            pt = pst4.tile([P, P], BF16, name="pt4")
                            nc.tensor.transpose(
                                pt[:], z_sb[:, qt, ec * P:(ec + 1) * P],
                                ident_sb)
                            nc.scalar.copy(
                                zT_sb[:, ec, qt * P:(qt + 1) * P], pt[:])

            # ---------- Stage 5: FFN + LN3 + output ----------
            with tc.tile_pool(name="st5", bufs=2) as st5, \
                 tc.tile_pool(name="lnp5", bufs=2) as lnp5, \
                 tc.tile_pool(name="igp", bufs=1) as igp:
                b1p_sb = st5.tile([P, F // P, 1], F32, name="b1p")
                nc.sync.dma_start(b1p_sb[:], b1p[:, :, None])
                if b2_nz:
                    b2r_sb = st5.tile([P, D], F32, name="b2r")
                    nc.sync.dma_start(b2r_sb[:], b2_bc)
                ig_sb = igp.tile([P, F // P, SQ], BF16, name="ig")
                w1_t = w1p.tile([P, 6, F], BF16, name="w1_t")
                for cc in range(6):
                    for sh_ in range(2):
                        nc.sync.dma_start(
                            w1_t[:, cc, sh_ * 1536:(sh_ + 1) * 1536],
                            w1.rearrange("(c p) e -> p c e", p=P)[
                                :, cc, sh_ * 1536:(sh_ + 1) * 1536])
                w2_t = w2p.tile([P, F // P, D], BF16, name="w2_t")
                for fq in range(12):
                    nc.sync.dma_start(
                        w2_t[:, fq * 2:(fq + 1) * 2, :],
                        w2.rearrange("(c p) e -> p c e", p=P)[
                            :, fq * 2:(fq + 1) * 2, :])
                with tc.tile_pool(name="ps5", bufs=3, space="PSUM") as ps5, \
                     tc.tile_pool(name="pso5", bufs=2, space="PSUM") as pso5:
                    for fc in range(F // P):
                        ps = ps5.tile([P, SQ], F32, name="ps5t")
                        for cc in range(6):
                            nc.tensor.matmul(
                                ps[:], w1_t[:, cc, fc * P:(fc + 1) * P],
                                zT_sb[:, cc, :],
                                start=(cc == 0), stop=(cc == 5))
                        nc.scalar.activation(ig_sb[:, fc, :], ps[:], AF.Gelu,
                                             bias=b1p_sb[:, fc, 0:1])
                    for qt in range(4):
                        po5 = pso5.tile([P, D], F32, name="po5")
                        for fc in range(F // P):
                            nc.tensor.matmul(
                                po5[:, 0:512],
                                ig_sb[:, fc, qt * P:(qt + 1) * P],
                                w2_t[:, fc, 0:512],
                                start=(fc == 0), stop=(fc == F // P - 1))
                            nc.tensor.matmul(
                                po5[:, 512:D],
                                ig_sb[:, fc, qt * P:(qt + 1) * P],
                                w2_t[:, fc, 512:D],
                                start=(fc == 0), stop=(fc == F // P - 1))
                        r5 = st5.tile([P, D], F32, name="r5")
                        nc.vector.tensor_add(r5[:], z_sb[:, qt, :], po5[:])
                        if b2_nz:
                            nc.vector.tensor_add(r5[:], r5[:], b2r_sb[:])
                        mv = _ln_mv(nc, lnp5, r5[:])
                        var = lnp5.tile([P, 1], F32, name="var5")
                        nc.vector.tensor_scalar(out=var[:], in0=mv[:, 1:2],
                                                scalar1=EPS, scalar2=None,
                                                op0=ALU.add)
                        rs = _rsqrt1(nc, lnp5, var[:], magic)
                        o_sb = lnp5.tile([P, D], F32, name="o5")
                        ln_apply(lnp5, r5[:], mv, rs, o_sb[:], 2)
                        nc.sync.dma_start(out[qt * P:(qt + 1) * P, :],
                                          o_sb[:])

    nc.compile()
    return nc


def _prep_shared(inp):
    """Host-side shared (core-independent) arrays."""
    f32, bf = np.float32, ml_dtypes.bfloat16
    sh = {}
    sh["wq"] = np.ascontiguousarray((inp["sa_wq"] * 0.125).astype(bf))
    sh["bq"] = np.ascontiguousarray(inp["sa_bq"] * 0.125)
    sh["wk"] = np.ascontiguousarray(inp["sa_wk"].astype(bf))
    sh["bk"] = np.ascontiguousarray(inp["sa_bk"])

    def aug(wvm, bvm):
        wva = np.zeros((D, DA), f32)
        bva = np.zeros((DA,), f32)
        for h in range(H):
            wva[:, h * HA:h * HA + DH] = wvm[:, h * DH:(h + 1) * DH]
            bva[h * HA:h * HA + DH] = bvm[h * DH:(h + 1) * DH]
            bva[h * HA + DH] = 1.0
        return wva, bva

    wva, bva = aug(inp["sa_wv"], inp["sa_bv"])
    sh["wv"] = np.ascontiguousarray(wva.astype(bf))
    sh["bv_bc"] = np.ascontiguousarray(
        np.broadcast_to(bva, (P, DA)).astype(bf))
    sh["wo"] = np.ascontiguousarray(inp["sa_wo"].astype(bf))
    sh["tagT"] = np.ascontiguousarray(inp["tag_emb"].T.astype(bf))
    sh["cwq"] = np.ascontiguousarray((inp["ca_wq"] * 0.125).astype(bf))
    sh["cbq"] = np.ascontiguousarray(inp["ca_bq"] * 0.125)
    sh["cwk"] = np.ascontiguousarray(inp["ca_wk"].astype(bf))
    sh["cbk"] = np.ascontiguousarray(inp["ca_bk"])
    cwva, cbva = aug(inp["ca_wv"], inp["ca_bv"])
    sh["cwv"] = np.ascontiguousarray(cwva.astype(bf))
    sh["cbv_bc"] = np.ascontiguousarray(
        np.broadcast_to(cbva, (T, DA)).astype(bf))
    sh["cwo"] = np.ascontiguousarray(inp["ca_wo"].astype(bf))
    sh["w1"] = np.ascontiguousarray(inp["ff_w1"].astype(bf))
    sh["b1p"] = np.ascontiguousarray(inp["ff_b1"].reshape(F // P, P).T)
    sh["w2"] = np.ascontiguousarray(inp["ff_w2"].astype(bf))
    sh["ident"] = np.eye(P, dtype=f32).astype(bf)
    o64 = np.zeros((65, 64), f32)
    o64[64, :] = 1.0
    sh["ones_in"] = np.ascontiguousarray(o64.astype(bf))
    return sh


def _mask5_for(qc):
    q0 = qc * SQ
    pos = np.arange(5 * P)
    s_true = (pos - 64 + q0) % S
    u = np.arange(SQ)
    band = (np.abs((q0 + u)[None, :] - s_true[:, None]) <= RAD)
    bexp = np.where(band, np.float32(np.e), np.float32(1.0)).astype(np.float32)
    bexp = bexp.reshape(5, P, SQ).transpose(1, 0, 2)  # [P, 5, SQ]
    packed = np.empty((P, BAND_TOT), ml_dtypes.bfloat16)
    for j, (lo, hi) in enumerate(BAND_COLS):
        packed[:, BAND_OFF[j]:BAND_OFF[j] + hi - lo] = bexp[:, j, lo:hi]
    return np.ascontiguousarray(packed)


def _make_in_maps(inp, aff, cbo_nz, b2_nz):
    sh = _prep_shared(inp)
    bf = ml_dtypes.bfloat16
    if cbo_nz:
        sh["cbo_bc"] = np.ascontiguousarray(
            np.broadcast_to(inp["ca_bo"], (P, D)).astype(np.float32))
    if b2_nz:
        sh["b2_bc"] = np.ascontiguousarray(
            np.broadcast_to(inp["ff_b2"], (P, D)).astype(np.float32))
    if aff:
        for k, src in (("g1_bc", "sa_ln_g"), ("b1l_bc", "sa_ln_b"),
                       ("g2_bc", "ca_ln_g"), ("b2l_bc", "ca_ln_b"),
                       ("g3_bc", "ff_ln_g"), ("b3l_bc", "ff_ln_b")):
            sh[k] = np.ascontiguousarray(
                np.broadcast_to(inp[src], (P, D)).astype(np.float32))
    masks = [_mask5_for(qc) for qc in range(4)]
    hs = inp["hidden_states"]
    in_maps = []
    for c in range(NC):
        b, qc = c // 4, c % 4
        q0 = qc * SQ
        xTb = hs[b].T
        m = dict(sh)
        m["xT"] = np.ascontiguousarray(
            np.roll(xTb, 64 - q0, axis=1).astype(bf))
        m["xres"] = np.ascontiguousarray(hs[b, q0:q0 + SQ] + inp["sa_bo"])
        m["mask5"] = masks[qc]
        in_maps.append(m)
    return in_maps


def kernel(**inputs):
    inp = {k: np.asarray(v, dtype=np.float32) for k, v in inputs.items()}
    aff = not (np.all(inp["sa_ln_g"] == 1) and np.all(inp["sa_ln_b"] == 0)
               and np.all(inp["ca_ln_g"] == 1) and np.all(inp["ca_ln_b"] == 0)
               and np.all(inp["ff_ln_g"] == 1) and np.all(inp["ff_ln_b"] == 0))
    cbo_nz = bool(np.any(inp["ca_bo"] != 0))
    b2_nz = bool(np.any(inp["ff_b2"] != 0))
    key = (aff, cbo_nz, b2_nz)
    if key not in _CACHED:
        _CACHED[key] = build_kernel(aff, cbo_nz, b2_nz)
    nc = _CACHED[key]

    in_maps = _make_in_maps(inp, aff, cbo_nz, b2_nz)
    res = bass_utils.run_bass_kernel_spmd(nc, in_maps, core_ids=list(range(NC)))
    out = np.empty((B, S, D), np.float32)
    for c in range(NC):
        b, qc = c // 4, c % 4
        out[b, qc * SQ:(qc + 1) * SQ] = res.results[c]["out"]
    return out
